# revision 16
# baseline (speedup 1.0000x reference)
"""Trainium2 Bass kernel v5 for AttentionAggregator (B=4, S=2048, H=1024, 16 heads).

Sharding (L-balanced): core c handles heads (2c, 2c+1) of EVERY batch; all
cores run the identical program, per-batch loop bounds specialized to L.

Transposed attention per (batch, head, query-half) unit:
  scoresT[kpos, q] = kT_tile.T @ q ; ET = exp(scoresT/8) -> f16 SBUF
  EV[d|den, q] += [vpos | ones].T @ ET   (PSUM, accumulated over k-tiles)
Ones-column gives softmax denominators. Host does normalization/pool/Wo.

v5 pipeline refinements over v4:
  - EV matmuls trail the score matmuls by TWO k-tiles so they never wait on
    the exp on the in-order PE queue.
  - Projections/transposes are interleaved as fine-grained per-t steps (2 MMs
    each) instead of 16-MM bursts, keeping the ACT stream fed.
  - Input DMAs spread across engine queues; lead-in starts after only the
    first half of batch 0.
"""

import numpy as np

S = 2048
HDIM = 1024
B = 4
NCORES = 8


def _bounds(L):
    ktn = [max(1, -(-int(l) // 128)) for l in L]
    qch = []
    for l in L:
        l = int(l)
        qch.append([
            max(0, min(2, -(-min(l, 1024) // 512))),
            max(0, min(2, -(-(l - 1024) // 512))) if l > 1024 else 0,
        ])
    pch = [max(1, -(-(k * 128) // 512)) for k in ktn]
    qpch = [q[0] + q[1] for q in qch]
    return ktn, qch, pch, qpch


def _build_program(L, debug=False):
    import concourse.mybir as mybir
    from concourse import bacc, tile

    f32 = mybir.dt.float32
    f16 = mybir.dt.float16
    nc = bacc.Bacc("TRN2", target_bir_lowering=False, debug=debug)

    KTN, QCH, PCH, QPCH = _bounds(L)

    xt_d = nc.dram_tensor("xt", [B, 2, 128, 8, 1024], f16, kind="ExternalInput")
    wq_d = nc.dram_tensor("wq", [128, 8, 128], f16, kind="ExternalInput")
    wk_d = nc.dram_tensor("wk", [128, 8, 128], f16, kind="ExternalInput")
    wv_d = nc.dram_tensor("wv", [128, 8, 128], f16, kind="ExternalInput")
    id_d = nc.dram_tensor("ident", [128, 128], f16, kind="ExternalInput")
    ev_d = nc.dram_tensor("ev", [B, 2, 2, 65, 1024], f32, kind="ExternalOutput")

    with tile.TileContext(nc) as tc:
        with (
            tc.tile_pool(name="const", bufs=1) as const,
            tc.tile_pool(name="xp", bufs=2) as xp,
            tc.tile_pool(name="qk", bufs=1) as qk,
            tc.tile_pool(name="vtp", bufs=2) as vtp,
            tc.tile_pool(name="etp", bufs=8) as etp,
            tc.tile_pool(name="evs", bufs=2) as evs,
            tc.tile_pool(name="ring", bufs=2, space="PSUM") as ring,
            tc.tile_pool(name="evp", bufs=1, space="PSUM") as evp,
            tc.tile_pool(name="pjp", bufs=1, space="PSUM") as pjp,
        ):
            wsb = {
                nm: const.tile([128, 8, 128], f16, name=f"{nm}_sb")
                for nm in ("wq", "wk", "wv")
            }
            idsb = const.tile([128, 128], f16, name="idsb")

            xsb = {}

            def dma_x(b, halves=(0, 1)):
                if b not in xsb:
                    xsb[b] = xp.tile([128, 8, S], f16, name="xtile")
                nch = max(PCH[b], QPCH[b])
                engs = [nc.sync, nc.gpsimd, nc.scalar] if b == 0 else [nc.sync]
                shards = [(0, 3), (3, 6), (6, 8)] if b == 0 else [(0, 8)]
                for half in halves:
                    lo, hi = half * 1024, min(nch * 512, (half + 1) * 1024)
                    if hi <= lo:
                        continue
                    for qi, (tl, th) in enumerate(shards):
                        engs[qi % len(engs)].dma_start(
                            out=xsb[b][:, tl:th, lo:hi],
                            in_=xt_d[b, half, :, tl:th, 0:hi - lo],
                        )

            # batch-0 first half + ident first so the lead-in starts fast
            nc.scalar.dma_start(out=idsb[:], in_=id_d[:])
            dma_x(0, halves=(0,))
            # warm the PE clock gate throughout the x DMA window: each burst
            # waits for one x shard to land, keeping the activity monitor busy
            wu = pjp.tile([128, 1024], f32, tag="pj", name="warmup")
            for t in range(8):
                for i in range(8):
                    nc.tensor.matmul(
                        wu[:, 0:128], idsb[:], xsb[0][:, t, 0:128],
                        start=True, stop=True,
                    )
            nc.scalar.dma_start(out=wsb["wq"][:], in_=wq_d[:])
            nc.scalar.dma_start(out=wsb["wk"][:], in_=wk_d[:])
            nc.scalar.dma_start(out=wsb["wv"][:], in_=wv_d[:])
            dma_x(0, halves=(1,))
            dma_x(1)

            qt = [qk.tile([128, S], f16, name=f"qt{b}") for b in range(B)]
            ktE = [qk.tile([128, S], f16, name=f"ktE{b}") for b in range(B)]
            ktO = [qk.tile([128, S], f16, name=f"ktO{b}") for b in range(B)]
            vpos = [qk.tile([128, 16, 129], f16, name=f"vpos{b}") for b in range(B)]
            for b in range(B):
                nc.vector.memset(ktE[b][64:128, :], 0.0)
                nc.vector.memset(ktO[b][0:64, :], 0.0)
            vt_cur = {}

            # ---------- fine-grained projection / transpose steps ----------
            def proj_steps(nm, b, half, dst_fn):
                """Yield per-t matmul steps + a final copy step for one
                1024-column half of a projection."""
                nch = PCH[b] if nm != "wq" else QPCH[b]
                ccs = [c for c in (0, 1) if half * 2 + c < nch]
                if not ccs:
                    return
                cell = {}

                def step_t(t):
                    def emit():
                        if t == 0:
                            cell["ps"] = pjp.tile(
                                [128, 1024], f32, tag="pj", name="proj_ps"
                            )
                        ps = cell["ps"]
                        for cc in ccs:
                            o = half * 1024 + cc * 512
                            nc.tensor.matmul(
                                ps[:, cc * 512:(cc + 1) * 512],
                                wsb[nm][:, t],
                                xsb[b][:, t, o:o + 512],
                                start=(t == 0),
                                stop=(t == 7),
                            )
                    return emit

                for t in range(8):
                    yield step_t(t)

                def copy_step():
                    ps = cell["ps"]
                    dst = dst_fn()
                    for cc in ccs:
                        o = half * 1024 + cc * 512
                        if nm == "wk":
                            nc.vector.tensor_copy(
                                dst[0][0:64, o:o + 512],
                                ps[0:64, cc * 512:(cc + 1) * 512],
                            )
                            nc.vector.tensor_copy(
                                dst[1][64:128, o:o + 512],
                                ps[64:128, cc * 512:(cc + 1) * 512],
                            )
                        else:
                            nc.vector.tensor_copy(
                                dst[:, o:o + 512], ps[:, cc * 512:(cc + 1) * 512]
                            )
                yield copy_step

            def v_alloc(b):
                def emit():
                    vt_cur[b] = vtp.tile([128, S], f16, name="vt")
                return emit

            def transpose_steps(b, grp):
                kts = list(range(grp * 8, min((grp + 1) * 8, KTN[b])))
                if not kts:
                    return
                cell = {}

                def tstep(sub):
                    def emit():
                        if sub == 0:
                            cell["tps"] = pjp.tile(
                                [128, 8, 128], f16, tag="pj", name="tps"
                            )
                        tps = cell["tps"]
                        for i in range(sub * 4, min((sub + 1) * 4, len(kts))):
                            nc.tensor.transpose(
                                tps[:, i],
                                vt_cur[b][:, kts[i] * 128:(kts[i] + 1) * 128],
                                idsb[:],
                            )
                    return emit

                yield tstep(0)
                if len(kts) > 4:
                    yield tstep(1)

                def copy_step():
                    tps = cell["tps"]
                    n = len(kts)
                    dst = vpos[b]
                    nc.vector.tensor_copy(
                        dst[:, kts[0]:kts[0] + n, 0:64], tps[:, 0:n, 0:64]
                    )
                    nc.vector.tensor_copy(
                        dst[:, kts[0]:kts[0] + n, 65:129], tps[:, 0:n, 64:128]
                    )
                    nc.vector.memset(dst[:, kts[0]:kts[0] + n, 64:65], 1.0)
                yield copy_step

            def batch_steps(b, first):
                """first: the part needed before attention on b can start
                (half 0 of q/k/v + transpose grp 0); rest comes via fill."""
                steps = []
                if first:
                    steps.extend(proj_steps("wq", b, 0, lambda b=b: qt[b]))
                    steps.extend(proj_steps("wk", b, 0, lambda b=b: (ktE[b], ktO[b])))
                    steps.append(v_alloc(b))
                    steps.extend(proj_steps("wv", b, 0, lambda b=b: vt_cur[b]))
                    steps.extend(transpose_steps(b, 0))
                else:
                    steps.extend(proj_steps("wk", b, 1, lambda b=b: (ktE[b], ktO[b])))
                    steps.extend(proj_steps("wv", b, 1, lambda b=b: vt_cur[b]))
                    steps.extend(transpose_steps(b, 1))
                    steps.extend(proj_steps("wq", b, 1, lambda b=b: qt[b]))
                return steps

            # lead-in: only q/k first halves of batch 0 emitted contiguously;
            # v+transposes land in the first two iterations via the fill
            lead = []
            lead.extend(proj_steps("wq", 0, 0, lambda: qt[0]))
            lead.extend(proj_steps("wk", 0, 0, lambda: (ktE[0], ktO[0])))
            for s in lead:
                s()

            def due_range(steps, lo, hi):
                n = max(1, len(steps))
                hi = max(hi, lo + 1)
                return [
                    (lo + (hi - lo) * i // n, s) for i, s in enumerate(steps)
                ]

            fill = []
            vpart = [v_alloc(0)]
            vpart.extend(proj_steps("wv", 0, 0, lambda: vt_cur[0]))
            vpart.extend(transpose_steps(0, 0))
            fill += due_range(vpart, 0, 4)           # EV(kt0) pops late (thr 6)
            rest0 = batch_steps(0, False)
            fill += due_range(rest0, 2, 14)          # k/v h1 by kt 8, q h1 by 16
            # per-batch unit-start iterations
            ustart = [0]
            for b in range(B):
                ustart.append(
                    ustart[-1]
                    + KTN[b] * sum(2 for q in range(2) if QCH[b][q] > 0)
                )
            for b in range(1, B):
                lo = ustart[b - 1] + (18 if b == 1 else 2)
                fill += due_range(batch_steps(b, True), lo, ustart[b] - 12)
                fill += due_range(
                    batch_steps(b, False), ustart[b] - 8, ustart[b] + 10
                )
            dma_x(2)
            dma_x(3)
            fill.sort(key=lambda p: p[0])
            fidx = [0]
            done = [False]

            def pull_due(force=False):
                while fidx[0] < len(fill) and (
                    force or fill[fidx[0]][0] <= giter[0]
                ):
                    fill[fidx[0]][1]()
                    fidx[0] += 1
                if fidx[0] >= len(fill):
                    done[0] = True

            giter = [0]
            pend = []  # deferred PE/DVE work, drained ~2 iterations later

            def tick():
                giter[0] += 1
                pull_due()
                thr = 6 if giter[0] < 12 else 2
                while len(pend) > thr:
                    pend.pop(0)()

            # ---------- attention units ----------
            def unit(b, hh, qh):
                nq = QCH[b][qh]
                if nq == 0:
                    return
                lo = hh * 64
                evt = evp.tile([65, 1024], f32, tag="ev", name="evps")
                ktn = KTN[b]

                def ev_mm(ktile, et_t):
                    def emit():
                        vsl = vpos[b][:, ktile, lo:lo + 65]
                        for cc in range(nq):
                            nc.tensor.matmul(
                                evt[:, cc * 512:(cc + 1) * 512],
                                vsl,
                                et_t[:, cc * 512:(cc + 1) * 512],
                                start=(ktile == 0),
                                stop=(ktile == ktn - 1),
                            )
                    return emit

                def writeout():
                    def emit():
                        stage = evs.tile([65, 1024], f32, name="evstage")
                        nc.vector.tensor_copy(stage[:], evt[:])
                        nc.gpsimd.dma_start(out=ev_d[b, hh, qh], in_=stage[:])
                    return emit

                for ktile in range(ktn):
                    ring_t = ring.tile([128, 1024], f32, tag="ring", name="sc_ps")
                    ktz = ktE[b] if hh == 0 else ktO[b]
                    for cc in range(nq):
                        nc.tensor.matmul(
                            ring_t[:, cc * 512:(cc + 1) * 512],
                            ktz[:, ktile * 128:(ktile + 1) * 128],
                            qt[b][:,
                                  qh * 1024 + cc * 512:qh * 1024 + (cc + 1) * 512],
                            start=True,
                            stop=True,
                        )
                    et_t = etp.tile([128, 1024], f16, name="et")
                    nc.scalar.activation(
                        out=et_t[:, 0:nq * 512],
                        in_=ring_t[:, 0:nq * 512],
                        func=mybir.ActivationFunctionType.Exp,
                        scale=0.125,
                    )
                    pend.append(ev_mm(ktile, et_t))
                    tick()
                pend.append(writeout())

            for b in range(B):
                for hh in range(2):
                    for qh in range(2):
                        unit(b, hh, qh)
            for p in pend:
                p()
            if not done[0]:
                pull_due(force=True)

    nc.compile()
    return nc


def _make_in_maps(x, L, Wq, Wk, Wv):
    x = np.asarray(x, dtype=np.float32)
    L = np.asarray(L)
    ident = np.eye(128, dtype=np.float16)
    xt = np.empty((B, 2, 128, 8, 1024), dtype=np.float16)
    for b in range(B):
        smask = (np.arange(S) < int(L[b])).astype(np.float32)
        xb = (x[b].T * smask[None, :]).reshape(8, 128, 2, 1024).astype(np.float16)
        xt[b] = xb.transpose(2, 1, 0, 3)
    in_maps = []
    for core in range(NCORES):
        m = {"xt": xt, "ident": ident}
        for nm, W in (("wq", Wq), ("wk", Wk), ("wv", Wv)):
            ws = np.asarray(W, dtype=np.float32)[core * 128:(core + 1) * 128, :].T
            m[nm] = np.ascontiguousarray(
                ws.reshape(8, 128, 128).transpose(1, 0, 2), dtype=np.float16
            )
        in_maps.append(m)
    return in_maps


def _postprocess(results, L, bv, Wo, bo):
    L = np.asarray(L)
    KTN, QCH, _, _ = _bounds(L)
    pooled = np.zeros((B, HDIM), dtype=np.float32)
    for core in range(NCORES):
        ev = np.asarray(results[core]["ev"])  # [B, 2, 2, 65, 1024]
        for b in range(B):
            Lb = int(L[b])
            for hh in range(2):
                cols = []
                for qh in range(2):
                    nq = QCH[b][qh]
                    if nq:
                        cols.append(ev[b, hh, qh][:, :nq * 512])
                flat = np.concatenate(cols, axis=1)
                ncols = flat.shape[1]
                if hh == 0:
                    dims, den = flat[0:64], flat[64]
                else:
                    den, dims = flat[0], flat[1:65]
                den_true = den - np.float32(KTN[b] * 128 - Lb)
                valid = np.arange(ncols) < Lb
                r = np.where(valid, 1.0 / (Lb * den_true), 0.0).astype(np.float32)
                g = core * 2 + hh
                pooled[b, g * 64:(g + 1) * 64] = dims @ r
    pooled = pooled + np.asarray(bv, dtype=np.float32)[None, :]
    out = pooled @ np.asarray(Wo, dtype=np.float32).T + np.asarray(bo, np.float32)
    return out.astype(np.float32)


_RUN_KWARGS = {}


def kernel(x, L, Wq, Wk, Wv, bv, Wo, bo):
    from concourse.bass_utils import run_bass_kernel_spmd

    nc = _build_program(np.asarray(L))
    in_maps = _make_in_maps(x, L, Wq, Wk, Wv)
    res = run_bass_kernel_spmd(nc, in_maps, list(range(NCORES)), **_RUN_KWARGS)
    kernel.last_results = res
    return _postprocess(res.results, L, bv, Wo, bo)


# revision 19
# speedup vs baseline: 1.0078x; 1.0078x over previous
"""Trainium2 Bass kernel v5 for AttentionAggregator (B=4, S=2048, H=1024, 16 heads).

Sharding (L-balanced): core c handles heads (2c, 2c+1) of EVERY batch; all
cores run the identical program, per-batch loop bounds specialized to L.

Transposed attention per (batch, head, query-half) unit:
  scoresT[kpos, q] = kT_tile.T @ q ; ET = exp(scoresT/8) -> f16 SBUF
  EV[d|den, q] += [vpos | ones].T @ ET   (PSUM, accumulated over k-tiles)
Ones-column gives softmax denominators. Host does normalization/pool/Wo.

v5 pipeline refinements over v4:
  - EV matmuls trail the score matmuls by TWO k-tiles so they never wait on
    the exp on the in-order PE queue.
  - Projections/transposes are interleaved as fine-grained per-t steps (2 MMs
    each) instead of 16-MM bursts, keeping the ACT stream fed.
  - Input DMAs spread across engine queues; lead-in starts after only the
    first half of batch 0.
"""

import numpy as np

S = 2048
HDIM = 1024
B = 4
NCORES = 8


def _bounds(L):
    ktn = [max(1, -(-int(l) // 128)) for l in L]
    qch = []
    for l in L:
        l = int(l)
        qch.append([
            max(0, min(2, -(-min(l, 1024) // 512))),
            max(0, min(2, -(-(l - 1024) // 512))) if l > 1024 else 0,
        ])
    pch = [max(1, -(-(k * 128) // 512)) for k in ktn]
    qpch = [q[0] + q[1] for q in qch]
    return ktn, qch, pch, qpch


def _build_program(L, debug=False):
    import concourse.mybir as mybir
    from concourse import bacc, tile

    f32 = mybir.dt.float32
    f16 = mybir.dt.float16
    nc = bacc.Bacc("TRN2", target_bir_lowering=False, debug=debug)

    KTN, QCH, PCH, QPCH = _bounds(L)

    xt_d = nc.dram_tensor("xt", [B, 2, 128, 8, 1024], f16, kind="ExternalInput")
    wq_d = nc.dram_tensor("wq", [128, 8, 128], f16, kind="ExternalInput")
    wk_d = nc.dram_tensor("wk", [128, 8, 128], f16, kind="ExternalInput")
    wv_d = nc.dram_tensor("wv", [128, 8, 128], f16, kind="ExternalInput")
    id_d = nc.dram_tensor("ident", [128, 128], f16, kind="ExternalInput")
    ev_d = nc.dram_tensor("ev", [B, 2, 2, 65, 1024], f32, kind="ExternalOutput")

    with tile.TileContext(nc) as tc:
        with (
            tc.tile_pool(name="const", bufs=1) as const,
            tc.tile_pool(name="xp", bufs=2) as xp,
            tc.tile_pool(name="qk", bufs=1) as qk,
            tc.tile_pool(name="vtp", bufs=2) as vtp,
            tc.tile_pool(name="etp", bufs=6) as etp,
            tc.tile_pool(name="evs", bufs=2) as evs,
            tc.tile_pool(name="ring", bufs=2, space="PSUM") as ring,
            tc.tile_pool(name="evp", bufs=1, space="PSUM") as evp,
            tc.tile_pool(name="pjp", bufs=1, space="PSUM") as pjp,
        ):
            wsb = {
                nm: const.tile([128, 8, 128], f16, name=f"{nm}_sb")
                for nm in ("wq", "wk", "wv")
            }
            idsb = const.tile([128, 128], f16, name="idsb")

            xsb = {}

            def dma_x(b, halves=(0, 1)):
                if b not in xsb:
                    xsb[b] = xp.tile([128, 8, S], f16, name="xtile")
                nch = max(PCH[b], QPCH[b])
                engs = [nc.sync, nc.gpsimd, nc.scalar] if b == 0 else [nc.sync]
                shards = [(0, 3), (3, 6), (6, 8)] if b == 0 else [(0, 8)]
                for half in halves:
                    lo, hi = half * 1024, min(nch * 512, (half + 1) * 1024)
                    if hi <= lo:
                        continue
                    for qi, (tl, th) in enumerate(shards):
                        engs[qi % len(engs)].dma_start(
                            out=xsb[b][:, tl:th, lo:hi],
                            in_=xt_d[b, half, :, tl:th, 0:hi - lo],
                        )

            # batch-0 first half + ident first so the lead-in starts fast
            nc.scalar.dma_start(out=idsb[:], in_=id_d[:])
            dma_x(0, halves=(0,))
            # warm the PE clock gate throughout the x DMA window: each burst
            # waits for one x shard to land, keeping the activity monitor busy
            wu = pjp.tile([128, 1024], f32, tag="pj", name="warmup")
            for t in range(8):
                for i in range(8):
                    nc.tensor.matmul(
                        wu[:, 0:128], idsb[:], xsb[0][:, t, 0:128],
                        start=True, stop=True,
                    )
            nc.scalar.dma_start(out=wsb["wq"][:], in_=wq_d[:])
            nc.scalar.dma_start(out=wsb["wk"][:], in_=wk_d[:])
            nc.scalar.dma_start(out=wsb["wv"][:], in_=wv_d[:])
            dma_x(0, halves=(1,))
            dma_x(1)

            qt = [qk.tile([128, S], f16, name=f"qt{b}") for b in range(B)]
            ktE = [qk.tile([128, S], f16, name=f"ktE{b}") for b in range(B)]
            ktO = [qk.tile([128, S], f16, name=f"ktO{b}") for b in range(B)]
            vpos = [qk.tile([128, 16, 129], f16, name=f"vpos{b}") for b in range(B)]
            for b in range(B):
                nc.vector.memset(ktE[b][64:128, :], 0.0)
                nc.vector.memset(ktO[b][0:64, :], 0.0)
            vt_cur = {}

            # ---------- fine-grained projection / transpose steps ----------
            def proj_steps(nm, b, half, dst_fn):
                """Yield per-t matmul steps + a final copy step for one
                1024-column half of a projection."""
                nch = PCH[b] if nm != "wq" else QPCH[b]
                ccs = [c for c in (0, 1) if half * 2 + c < nch]
                if not ccs:
                    return
                cell = {}

                def step_t(t):
                    def emit():
                        if t == 0:
                            cell["ps"] = pjp.tile(
                                [128, 1024], f32, tag="pj", name="proj_ps"
                            )
                        ps = cell["ps"]
                        for cc in ccs:
                            o = half * 1024 + cc * 512
                            nc.tensor.matmul(
                                ps[:, cc * 512:(cc + 1) * 512],
                                wsb[nm][:, t],
                                xsb[b][:, t, o:o + 512],
                                start=(t == 0),
                                stop=(t == 7),
                            )
                    return emit

                for t in range(8):
                    yield step_t(t)

                def copy_step():
                    ps = cell["ps"]
                    dst = dst_fn()
                    for cc in ccs:
                        o = half * 1024 + cc * 512
                        if nm == "wk":
                            nc.vector.tensor_copy(
                                dst[0][0:64, o:o + 512],
                                ps[0:64, cc * 512:(cc + 1) * 512],
                            )
                            nc.vector.tensor_copy(
                                dst[1][64:128, o:o + 512],
                                ps[64:128, cc * 512:(cc + 1) * 512],
                            )
                        else:
                            nc.vector.tensor_copy(
                                dst[:, o:o + 512], ps[:, cc * 512:(cc + 1) * 512]
                            )
                yield copy_step

            def v_alloc(b):
                def emit():
                    vt_cur[b] = vtp.tile([128, S], f16, name="vt")
                return emit

            def transpose_steps(b, grp):
                kts = list(range(grp * 8, min((grp + 1) * 8, KTN[b])))
                if not kts:
                    return
                cell = {}

                def tstep(sub):
                    def emit():
                        if sub == 0:
                            cell["tps"] = pjp.tile(
                                [128, 8, 128], f16, tag="pj", name="tps"
                            )
                        tps = cell["tps"]
                        for i in range(sub * 4, min((sub + 1) * 4, len(kts))):
                            nc.tensor.transpose(
                                tps[:, i],
                                vt_cur[b][:, kts[i] * 128:(kts[i] + 1) * 128],
                                idsb[:],
                            )
                    return emit

                yield tstep(0)
                if len(kts) > 4:
                    yield tstep(1)

                def copy_step():
                    tps = cell["tps"]
                    n = len(kts)
                    dst = vpos[b]
                    nc.vector.tensor_copy(
                        dst[:, kts[0]:kts[0] + n, 0:64], tps[:, 0:n, 0:64]
                    )
                    nc.vector.tensor_copy(
                        dst[:, kts[0]:kts[0] + n, 65:129], tps[:, 0:n, 64:128]
                    )
                    nc.vector.memset(dst[:, kts[0]:kts[0] + n, 64:65], 1.0)
                yield copy_step

            def batch_steps(b, first):
                """first: the part needed before attention on b can start
                (half 0 of q/k/v + transpose grp 0); rest comes via fill."""
                steps = []
                if first:
                    steps.extend(proj_steps("wq", b, 0, lambda b=b: qt[b]))
                    steps.extend(proj_steps("wk", b, 0, lambda b=b: (ktE[b], ktO[b])))
                    steps.append(v_alloc(b))
                    steps.extend(proj_steps("wv", b, 0, lambda b=b: vt_cur[b]))
                    steps.extend(transpose_steps(b, 0))
                else:
                    steps.extend(proj_steps("wk", b, 1, lambda b=b: (ktE[b], ktO[b])))
                    steps.extend(proj_steps("wv", b, 1, lambda b=b: vt_cur[b]))
                    steps.extend(transpose_steps(b, 1))
                    steps.extend(proj_steps("wq", b, 1, lambda b=b: qt[b]))
                return steps

            # lead-in: only q/k first halves of batch 0 emitted contiguously;
            # v+transposes land in the first two iterations via the fill
            lead = []
            lead.extend(proj_steps("wq", 0, 0, lambda: qt[0]))
            lead.extend(proj_steps("wk", 0, 0, lambda: (ktE[0], ktO[0])))
            for s in lead:
                s()

            def due_range(steps, lo, hi):
                n = max(1, len(steps))
                hi = max(hi, lo + 1)
                return [
                    (lo + (hi - lo) * i // n, s) for i, s in enumerate(steps)
                ]

            fill = []
            vpart = [v_alloc(0)]
            vpart.extend(proj_steps("wv", 0, 0, lambda: vt_cur[0]))
            vpart.extend(transpose_steps(0, 0))
            fill += due_range(vpart, 0, 1)           # needed by EV(kt0) ~iter 2
            rest0 = batch_steps(0, False)
            fill += due_range(rest0, 2, 14)          # k/v h1 by kt 8, q h1 by 16
            # per-batch unit-start iterations
            ustart = [0]
            for b in range(B):
                ustart.append(
                    ustart[-1]
                    + KTN[b] * sum(2 for q in range(2) if QCH[b][q] > 0)
                )
            for b in range(1, B):
                lo = ustart[b - 1] + (18 if b == 1 else 2)
                fill += due_range(batch_steps(b, True), lo, ustart[b] - 12)
                fill += due_range(
                    batch_steps(b, False), ustart[b] - 8, ustart[b] + 10
                )
            dma_x(2)
            dma_x(3)
            fill.sort(key=lambda p: p[0])
            fidx = [0]
            done = [False]

            def pull_due(force=False):
                while fidx[0] < len(fill) and (
                    force or fill[fidx[0]][0] <= giter[0]
                ):
                    fill[fidx[0]][1]()
                    fidx[0] += 1
                if fidx[0] >= len(fill):
                    done[0] = True

            giter = [0]
            pend = []  # deferred PE/DVE work, drained ~2 iterations later

            def tick():
                giter[0] += 1
                pull_due()
                while len(pend) > 2:
                    pend.pop(0)()

            # ---------- attention units ----------
            def unit(b, hh, qh):
                nq = QCH[b][qh]
                if nq == 0:
                    return
                lo = hh * 64
                evt = evp.tile([65, 1024], f32, tag="ev", name="evps")
                ktn = KTN[b]

                def ev_mm(ktile, et_t):
                    def emit():
                        vsl = vpos[b][:, ktile, lo:lo + 65]
                        for cc in range(nq):
                            nc.tensor.matmul(
                                evt[:, cc * 512:(cc + 1) * 512],
                                vsl,
                                et_t[:, cc * 512:(cc + 1) * 512],
                                start=(ktile == 0),
                                stop=(ktile == ktn - 1),
                            )
                    return emit

                def writeout():
                    def emit():
                        stage = evs.tile([65, 1024], f32, name="evstage")
                        nc.vector.tensor_copy(stage[:], evt[:])
                        nc.gpsimd.dma_start(out=ev_d[b, hh, qh], in_=stage[:])
                    return emit

                for ktile in range(ktn):
                    ring_t = ring.tile([128, 1024], f32, tag="ring", name="sc_ps")
                    ktz = ktE[b] if hh == 0 else ktO[b]
                    for cc in range(nq):
                        nc.tensor.matmul(
                            ring_t[:, cc * 512:(cc + 1) * 512],
                            ktz[:, ktile * 128:(ktile + 1) * 128],
                            qt[b][:,
                                  qh * 1024 + cc * 512:qh * 1024 + (cc + 1) * 512],
                            start=True,
                            stop=True,
                        )
                    et_t = etp.tile([128, 1024], f16, name="et")
                    nc.scalar.activation(
                        out=et_t[:, 0:nq * 512],
                        in_=ring_t[:, 0:nq * 512],
                        func=mybir.ActivationFunctionType.Exp,
                        scale=0.125,
                    )
                    pend.append(ev_mm(ktile, et_t))
                    tick()
                pend.append(writeout())

            for b in range(B):
                for hh in range(2):
                    for qh in range(2):
                        unit(b, hh, qh)
            for p in pend:
                p()
            if not done[0]:
                pull_due(force=True)

    nc.compile()
    return nc


def _make_in_maps(x, L, Wq, Wk, Wv):
    x = np.asarray(x, dtype=np.float32)
    L = np.asarray(L)
    ident = np.eye(128, dtype=np.float16)
    xt = np.empty((B, 2, 128, 8, 1024), dtype=np.float16)
    for b in range(B):
        smask = (np.arange(S) < int(L[b])).astype(np.float32)
        xb = (x[b].T * smask[None, :]).reshape(8, 128, 2, 1024).astype(np.float16)
        xt[b] = xb.transpose(2, 1, 0, 3)
    in_maps = []
    for core in range(NCORES):
        m = {"xt": xt, "ident": ident}
        for nm, W in (("wq", Wq), ("wk", Wk), ("wv", Wv)):
            ws = np.asarray(W, dtype=np.float32)[core * 128:(core + 1) * 128, :].T
            m[nm] = np.ascontiguousarray(
                ws.reshape(8, 128, 128).transpose(1, 0, 2), dtype=np.float16
            )
        in_maps.append(m)
    return in_maps


def _postprocess(results, L, bv, Wo, bo):
    L = np.asarray(L)
    KTN, QCH, _, _ = _bounds(L)
    pooled = np.zeros((B, HDIM), dtype=np.float32)
    for core in range(NCORES):
        ev = np.asarray(results[core]["ev"])  # [B, 2, 2, 65, 1024]
        for b in range(B):
            Lb = int(L[b])
            for hh in range(2):
                cols = []
                for qh in range(2):
                    nq = QCH[b][qh]
                    if nq:
                        cols.append(ev[b, hh, qh][:, :nq * 512])
                flat = np.concatenate(cols, axis=1)
                ncols = flat.shape[1]
                if hh == 0:
                    dims, den = flat[0:64], flat[64]
                else:
                    den, dims = flat[0], flat[1:65]
                den_true = den - np.float32(KTN[b] * 128 - Lb)
                valid = np.arange(ncols) < Lb
                r = np.where(valid, 1.0 / (Lb * den_true), 0.0).astype(np.float32)
                g = core * 2 + hh
                pooled[b, g * 64:(g + 1) * 64] = dims @ r
    pooled = pooled + np.asarray(bv, dtype=np.float32)[None, :]
    out = pooled @ np.asarray(Wo, dtype=np.float32).T + np.asarray(bo, np.float32)
    return out.astype(np.float32)


_RUN_KWARGS = {}


def kernel(x, L, Wq, Wk, Wv, bv, Wo, bo):
    from concourse.bass_utils import run_bass_kernel_spmd

    nc = _build_program(np.asarray(L))
    in_maps = _make_in_maps(x, L, Wq, Wk, Wv)
    res = run_bass_kernel_spmd(nc, in_maps, list(range(NCORES)), **_RUN_KWARGS)
    kernel.last_results = res
    return _postprocess(res.results, L, bv, Wo, bo)
